# revision 1
# baseline (speedup 1.0000x reference)
"""ChemConv Bass kernel for 8 TRN2 NeuronCores.

Math: the reference
    node_connection[a,f,i] = sum_n conn[a,n,f] * x[n,i]
    bond_score[a,o,f]      = sum_i node_connection[a,f,i] * pf[o,f,i]
    out[a,o] = sum_f bond_score[a,o,f]*bf[o,f,0] + sum_{f,c} bp[a,f,c]*bf[o,f,1+c]
collapses algebraically to one large matmul plus small ones:
    W[o,f,i]  = pf[o,f,i] * bf[o,f,0]
    Y[k=(f,n), o] = sum_i x[n,i] * W[o,f,i]          (tiny: 24576 x 64)
    out[a,o]  = sum_k conn2d[a,k] * Y[k,o] + sum_j bpT[j,a] * bf2[j,o]
where conn2d[a, (f,n)] = conn[a,n,f] (201 MB -> the memory-bound stream).

Sharding: atoms (dim a) row-slabs of 256 across 8 cores. Each core computes
out_T[o, a_slab] via PSUM accumulation over 192 K-chunks of 128.
conn is pre-transposed host-side to [K, a_slab] so DMA loads land with the
contraction dim on SBUF partitions (PE needs partition = K on both operands).
float32r streams the fp32 moving operand at ~1 cycle/row (vs 4 for fp32).
Y is computed on device from x^T (0.5 MB) instead of DMAing 6.3 MB.
"""

import numpy as np

import concourse.bass as bass
import concourse.tile as tile
from concourse import bacc, mybir
from concourse.bass_utils import run_bass_kernel_spmd

A = 2048
IN_DEPTH = 64
OUT_DEPTH = 64
F = 12
NCORES = 8
AS = A // NCORES          # 256 atoms per core
K = A * F                 # 24576 contraction length
KP = 128                  # K per matmul chunk (partition dim)
KC = K // KP              # 192 chunks
NBLK = A // KP            # 16 n-blocks per filter tap
KB = 2 * F                # bond-term contraction length (f,c) = 24
YG = 8                    # y chunks per PSUM bank group (8*64 = 512 = bank)

MM_DT = mybir.dt.float32r  # fp32 bits, full-rate PE streaming mode
F32 = mybir.dt.float32

_cache = {}


def _build_nc(repeat=1, B=32, bufs=4, y_dev=True, split_dma=False, y_ring=8):
    """Build the per-core kernel.

    repeat: re-run the whole body N times (benchmark-only; deliverable uses 1)
    B: K-chunks per DMA batch (B*128*256*4 bytes per transfer)
    bufs: stream-pool buffering depth
    y_dev: compute Y on device from xT/Wr (vs DMA the precomputed 6.3 MB)
    split_dma: issue each conn batch as two half-DMAs on the two HWDGE rings
    """
    NB = KC // B
    nc = bacc.Bacc("TRN2", target_bir_lowering=False, debug=False)

    conn_t = nc.dram_tensor("conn_t", [K, AS], MM_DT, kind="ExternalInput").ap()
    # bond_t [24, AS] and bf2 [24, O] packed side by side -> one DMA
    bpack = nc.dram_tensor("bpack", [KB, AS + OUT_DEPTH], F32, kind="ExternalInput").ap()
    if y_dev:
        # xT [64, A] and Wr [64, F*O] packed side by side -> one DMA
        xw = nc.dram_tensor("xw", [IN_DEPTH, A + F * OUT_DEPTH], F32,
                            kind="ExternalInput").ap()
    else:
        ypack = nc.dram_tensor("ypack", [KP, KC * OUT_DEPTH], MM_DT, kind="ExternalInput").ap()
        ydram_v = ypack.rearrange("p (nb o) -> p nb o", nb=KC)
    out_t = nc.dram_tensor("out_t", [OUT_DEPTH, AS], F32, kind="ExternalOutput").ap()

    # DRAM view of conn_t with the chunk partition dim innermost:
    # [K, AS] -> [p=128, nb=KC, a=AS]
    conn_v = conn_t.rearrange("(nb p) a -> p nb a", p=KP)

    # conn DMA batch sizes: big batches for bandwidth, tapered tail so the
    # final accumulating matmuls (which gate the output copy) start early
    batches = [B] * (KC // B - 1) + [B // 2, B // 4, B // 8, B // 8]
    assert sum(batches) == KC
    NG = KC // YG               # y groups total

    with tile.TileContext(nc) as tc:
        with (
            tc.tile_pool(name="const", bufs=1) as cpool,
            tc.tile_pool(name="ypool", bufs=(y_ring if y_dev else 1)) as ypool,
            tc.tile_pool(name="stream", bufs=bufs) as spool,
            tc.tile_pool(name="psum", bufs=2, space="PSUM") as ppool,
            tc.tile_pool(name="ypsum", bufs=2, space="PSUM") as ypp,
        ):
            # small input DMAs, packed, on the second HWDGE ring (ACT) so the
            # conn stream owns the SP ring from t=0
            bp_sb = cpool.tile([KB, AS + OUT_DEPTH], F32)
            nc.scalar.dma_start(bp_sb[:], bpack[:])
            bond_sb = bp_sb[:, :AS]
            bf2_sb = bp_sb[:, AS:AS + OUT_DEPTH]
            if y_dev:
                xw_sb = cpool.tile([IN_DEPTH, A + F * OUT_DEPTH], F32)
                nc.scalar.dma_start(xw_sb[:], xw[:])
                xt_sb = xw_sb[:, :A]
                wr_sb = xw_sb[:, A:A + F * OUT_DEPTH]

            for rep in range(repeat):
                ygroups = {}
                if not y_dev:
                    y_sb = ypool.tile([KP, KC * OUT_DEPTH], MM_DT, tag="y")
                    y_v = y_sb.rearrange("p (nb o) -> p nb o", nb=KC)

                def y_chunk_ap(kc):
                    if y_dev:
                        g, j = divmod(kc, YG)
                        return ygroups[g][:, j * OUT_DEPTH:(j + 1) * OUT_DEPTH]
                    return y_v[:, kc, :]

                def y_group(g):
                    # Y[kc=(f,nb)] chunk = xT[:, nb-block].T @ Wr[:, f-block];
                    # each chunk is consumed by exactly one conn matmul, so
                    # groups live in a small ring (ypool bufs) not a flat 6.3MB
                    yps = ypp.tile([KP, YG * OUT_DEPTH], F32, tag="yps")
                    for j in range(YG):
                        kc = g * YG + j
                        f, nb = divmod(kc, NBLK)
                        nc.tensor.matmul(
                            yps[:, j * OUT_DEPTH:(j + 1) * OUT_DEPTH],
                            xt_sb[:, nb * KP:(nb + 1) * KP],
                            wr_sb[:, f * OUT_DEPTH:(f + 1) * OUT_DEPTH],
                            start=(j == 0),
                            stop=(j == YG - 1),
                        )
                    yt = ypool.tile([KP, YG * OUT_DEPTH], MM_DT, tag="y",
                                    name=f"yt_{rep}_{g}")
                    nc.vector.tensor_copy(yt[:], yps[:].bitcast(MM_DT))
                    ygroups[g] = yt

                # issue the first two conn batch DMAs before anything else so
                # the SP ring streams from t=0
                ctiles = {}
                k0 = 0
                starts = []
                for bt, bsz in enumerate(batches):
                    starts.append(k0)
                    k0 += bsz
                pre_issue = 2

                def issue_conn(bt):
                    bsz = batches[bt]
                    ctile = spool.tile([KP, bsz * AS], MM_DT, tag="conn",
                                       name=f"conn_{rep}_{bt}")
                    ctv = ctile.rearrange("p (b a) -> p b a", b=bsz)
                    nc.sync.dma_start(
                        ctv[:], conn_v[:, starts[bt]:starts[bt] + bsz, :])
                    ctiles[bt] = ctv

                for bt in range(pre_issue):
                    issue_conn(bt)

                yg_done = 0
                if not y_dev:
                    for i in range(KC // B):
                        nc.sync.dma_start(y_v[:, i * B:(i + 1) * B, :],
                                          ydram_v[:, i * B:(i + 1) * B, :])

                acc = ppool.tile([OUT_DEPTH, AS], F32, tag="acc")

                # bond term opens the PSUM accumulation group
                nc.tensor.matmul(acc[:], bf2_sb[:], bond_sb[:], start=True, stop=False)

                for bt, bsz in enumerate(batches):
                    if y_dev:
                        # y groups needed by batch bt+1 (lookahead), before
                        # this batch's matmuls occupy PE
                        need = min(NG, -(-(starts[min(bt + 1, len(batches) - 1)]
                                           + batches[min(bt + 1, len(batches) - 1)]) // YG))
                        while yg_done < need:
                            y_group(yg_done)
                            yg_done += 1
                    for b in range(bsz):
                        kc = starts[bt] + b
                        nc.tensor.matmul(
                            acc[:],
                            y_chunk_ap(kc),
                            ctiles[bt][:, b, :],
                            start=False,
                            stop=(kc == KC - 1),
                        )
                    # prefetch next batch's DMA
                    nxt = bt + pre_issue
                    if nxt < len(batches):
                        issue_conn(nxt)

                out_sb = spool.tile([OUT_DEPTH, AS], F32, tag="osb")
                nc.vector.tensor_copy(out_sb[:], acc[:])
                nc.sync.dma_start(out_t[:], out_sb[:])

    nc.compile()
    return nc


def _prep(node_property_tensor, connectivity_tensor, bond_property_tensor,
          property_filters, bond_filters, y_dev=True):
    x = np.asarray(node_property_tensor, dtype=np.float32)
    conn = np.asarray(connectivity_tensor, dtype=np.float32)
    bp = np.asarray(bond_property_tensor, dtype=np.float32)
    pf = np.asarray(property_filters, dtype=np.float32)
    bf = np.asarray(bond_filters, dtype=np.float32)

    W = pf * bf[:, :, 0:1]                                # (O, F, I)
    wr = np.ascontiguousarray(W.transpose(2, 1, 0).reshape(IN_DEPTH, F * OUT_DEPTH))
    bf2 = np.ascontiguousarray(bf[:, :, 1:3].reshape(OUT_DEPTH, KB).T)  # (24, O)

    common = {}
    if y_dev:
        common["xw"] = np.ascontiguousarray(
            np.concatenate([x.T, wr], axis=1))      # (64, A + F*O)
        # k = (f, n) major: conn_t[k, a] = conn[a, n, f]
        connT = np.ascontiguousarray(conn.transpose(2, 1, 0))  # (F, A_n, A_a)
        connT2 = connT.reshape(K, A)
    else:
        Y = x @ wr                                        # (A, F*O), k = n*F+f
        Y2d = Y.reshape(A * F, OUT_DEPTH)
        common["ypack"] = np.ascontiguousarray(
            Y2d.reshape(KC, KP, OUT_DEPTH).transpose(1, 0, 2)
            .reshape(KP, KC * OUT_DEPTH))
        connT2 = np.ascontiguousarray(conn.reshape(A, K).T)  # (K, A)

    in_maps = []
    for c in range(NCORES):
        sl = slice(c * AS, (c + 1) * AS)
        bond_tc = bp[sl].reshape(AS, KB).T              # (24, AS)
        in_maps.append({
            "conn_t": np.ascontiguousarray(connT2[:, sl]),
            "bpack": np.ascontiguousarray(
                np.concatenate([bond_tc, bf2], axis=1)),  # (24, AS + O)
            **common,
        })
    return in_maps


def kernel(node_property_tensor, connectivity_tensor, bond_property_tensor,
           property_filters, bond_filters):
    in_maps = _prep(node_property_tensor, connectivity_tensor,
                    bond_property_tensor, property_filters, bond_filters)

    if "nc" not in _cache:
        _cache["nc"] = _build_nc()
    nc = _cache["nc"]

    res = run_bass_kernel_spmd(nc, in_maps, core_ids=list(range(NCORES)))

    out = np.empty((A, OUT_DEPTH), dtype=np.float32)
    for c in range(NCORES):
        out[c * AS:(c + 1) * AS, :] = res.results[c]["out_t"].T
    return out



# revision 2
# speedup vs baseline: 2.0741x; 2.0741x over previous
"""ChemConv Bass kernel for 8 TRN2 NeuronCores.

Math: the reference
    node_connection[a,f,i] = sum_n conn[a,n,f] * x[n,i]
    bond_score[a,o,f]      = sum_i node_connection[a,f,i] * pf[o,f,i]
    out[a,o] = sum_f bond_score[a,o,f]*bf[o,f,0] + sum_{f,c} bp[a,f,c]*bf[o,f,1+c]
collapses algebraically to one large matmul plus small ones:
    W[o,f,i]  = pf[o,f,i] * bf[o,f,0]
    Y[k=(f,n), o] = sum_i x[n,i] * W[o,f,i]          (tiny: 24576 x 64)
    out[a,o]  = sum_k conn2d[a,k] * Y[k,o] + sum_j bpT[j,a] * bf2[j,o]
where conn2d[a, (f,n)] is the memory-bound stream.

Sharding: atoms (dim a) row-slabs of 256 across 8 cores. Each core computes
out_T[o, a_slab] via PSUM accumulation over 192 K-chunks of 128.

conn is cast to bf16 host-side (rel err ~3e-3, well under the 2e-2 gate):
halves the HBM stream to 12.6 MB/core and runs the PE at 1 cycle/row at any
p-state (fp32 is 4 cycles/row; fp32r needs ap>=256 and full p-state).
Host packs conn as [128, KC*AS] so each DMA batch reads 16 KB contiguous per
partition. Y is computed on device from bf16 x^T/W (0.27 MB DMA vs 3.1 MB).
"""

import numpy as np
import ml_dtypes

import concourse.bass as bass
import concourse.tile as tile
from concourse import bacc, mybir
from concourse.bass_utils import run_bass_kernel_spmd

A = 2048
IN_DEPTH = 64
OUT_DEPTH = 64
F = 12
NCORES = 8
AS = A // NCORES          # 256 atoms per core
K = A * F                 # 24576 contraction length
KP = 128                  # K per matmul chunk (partition dim)
KC = K // KP              # 192 chunks
NBLK = A // KP            # 16 n-blocks per filter tap
KB = 2 * F                # bond-term contraction length (f,c) = 24
YG = 8                    # y chunks per PSUM bank group (8*64 = 512 = bank)

BF16 = mybir.dt.bfloat16
F32 = mybir.dt.float32
NP_BF16 = ml_dtypes.bfloat16

_cache = {}


def _build_nc(B=32, bufs=4, y_dev=True, alt_q=False, y_ring=8):
    """Build the per-core kernel.

    B: K-chunks per DMA batch (B*AS*2 bytes per partition per transfer)
    bufs: stream-pool buffering depth
    y_dev: compute Y on device from xT/Wr (vs DMA the precomputed 6.1 MB)
    alt_q: alternate conn batch DMAs between the SP and ACT HWDGE rings
    """
    nc = bacc.Bacc("TRN2", target_bir_lowering=False, debug=False)

    conn_t = nc.dram_tensor("conn_t", [KP, KC * AS], BF16, kind="ExternalInput").ap()
    # bond_t [24, AS] and bf2 [24, O] packed side by side -> one DMA
    bpack = nc.dram_tensor("bpack", [KB, AS + OUT_DEPTH], F32, kind="ExternalInput").ap()
    if y_dev:
        # xT [64, A] and Wr [64, F*O] packed side by side -> one DMA (bf16)
        xw = nc.dram_tensor("xw", [IN_DEPTH, A + F * OUT_DEPTH], BF16,
                            kind="ExternalInput").ap()
    else:
        ypack = nc.dram_tensor("ypack", [KP, KC * OUT_DEPTH], BF16, kind="ExternalInput").ap()
    out_t = nc.dram_tensor("out_t", [OUT_DEPTH, AS], F32, kind="ExternalOutput").ap()

    # conn DMA batch sizes: big batches for bandwidth, tapered tail so the
    # final accumulating matmuls (which gate the output copy) start early
    batches = [B] * (KC // B - 1) + [B // 2, B // 4, B // 8, B // 8]
    assert sum(batches) == KC
    NG = KC // YG               # y groups total

    with tile.TileContext(nc) as tc:
        with (
            tc.tile_pool(name="const", bufs=1) as cpool,
            tc.tile_pool(name="ypool", bufs=(y_ring if y_dev else 1)) as ypool,
            tc.tile_pool(name="stream", bufs=bufs) as spool,
            tc.tile_pool(name="psum", bufs=2, space="PSUM") as ppool,
            tc.tile_pool(name="ypsum", bufs=2, space="PSUM") as ypp,
        ):
            # small input DMAs, packed, on the second HWDGE ring (ACT) so the
            # conn stream owns the SP ring from t=0
            bp_sb = cpool.tile([KB, AS + OUT_DEPTH], F32)
            nc.scalar.dma_start(bp_sb[:], bpack[:])
            bond_sb = bp_sb[:, :AS]
            bf2_sb = bp_sb[:, AS:AS + OUT_DEPTH]
            if y_dev:
                xw_sb = cpool.tile([IN_DEPTH, A + F * OUT_DEPTH], BF16)
                nc.scalar.dma_start(xw_sb[:], xw[:])
                xt_sb = xw_sb[:, :A]
                wr_sb = xw_sb[:, A:A + F * OUT_DEPTH]

            ygroups = {}
            if not y_dev:
                y_sb = ypool.tile([KP, KC * OUT_DEPTH], BF16, tag="y")

            def y_chunk_ap(kc):
                if y_dev:
                    g, j = divmod(kc, YG)
                    return ygroups[g][:, j * OUT_DEPTH:(j + 1) * OUT_DEPTH]
                return y_sb[:, kc * OUT_DEPTH:(kc + 1) * OUT_DEPTH]

            def y_group(g):
                # Y[kc=(f,nb)] chunk = xT[:, nb-block].T @ Wr[:, f-block];
                # each chunk is consumed by exactly one conn matmul, so
                # groups live in a small ring (ypool bufs) not a flat 6 MB
                yps = ypp.tile([KP, YG * OUT_DEPTH], F32, tag="yps")
                for j in range(YG):
                    kc = g * YG + j
                    f, nb = divmod(kc, NBLK)
                    nc.tensor.matmul(
                        yps[:, j * OUT_DEPTH:(j + 1) * OUT_DEPTH],
                        xt_sb[:, nb * KP:(nb + 1) * KP],
                        wr_sb[:, f * OUT_DEPTH:(f + 1) * OUT_DEPTH],
                        start=(j == 0),
                        stop=(j == YG - 1),
                    )
                yt = ypool.tile([KP, YG * OUT_DEPTH], BF16, tag="y",
                                name=f"yt_{g}")
                nc.vector.tensor_copy(yt[:], yps[:])
                ygroups[g] = yt

            # issue the first two conn batch DMAs before anything else so
            # the SP ring streams from t=0
            ctiles = {}
            k0 = 0
            starts = []
            for bt, bsz in enumerate(batches):
                starts.append(k0)
                k0 += bsz
            pre_issue = 2

            def issue_conn(bt):
                bsz = batches[bt]
                ctile = spool.tile([KP, bsz * AS], BF16, tag="conn",
                                   name=f"conn_{bt}")
                q = nc.scalar if (alt_q and bt % 2 == 1) else nc.sync
                q.dma_start(
                    ctile[:], conn_t[:, starts[bt] * AS:(starts[bt] + bsz) * AS])
                ctiles[bt] = ctile

            for bt in range(pre_issue):
                issue_conn(bt)

            yg_done = 0
            if not y_dev:
                NB = KC // B
                for i in range(NB):
                    nc.scalar.dma_start(
                        y_sb[:, i * B * OUT_DEPTH:(i + 1) * B * OUT_DEPTH],
                        ypack[:, i * B * OUT_DEPTH:(i + 1) * B * OUT_DEPTH])

            acc = ppool.tile([OUT_DEPTH, AS], F32, tag="acc")

            # bond term opens the PSUM accumulation group
            nc.tensor.matmul(acc[:], bf2_sb[:], bond_sb[:], start=True, stop=False)

            for bt, bsz in enumerate(batches):
                if y_dev:
                    # y groups needed by batch bt+1 (lookahead), before
                    # this batch's matmuls occupy PE
                    need = min(NG, -(-(starts[min(bt + 1, len(batches) - 1)]
                                       + batches[min(bt + 1, len(batches) - 1)]) // YG))
                    while yg_done < need:
                        y_group(yg_done)
                        yg_done += 1
                for b in range(bsz):
                    kc = starts[bt] + b
                    nc.tensor.matmul(
                        acc[:],
                        y_chunk_ap(kc),
                        ctiles[bt][:, b * AS:(b + 1) * AS],
                        start=False,
                        stop=(kc == KC - 1),
                    )
                # prefetch next batch's DMA
                nxt = bt + pre_issue
                if nxt < len(batches):
                    issue_conn(nxt)

            out_sb = spool.tile([OUT_DEPTH, AS], F32, tag="osb")
            nc.vector.tensor_copy(out_sb[:], acc[:])
            nc.sync.dma_start(out_t[:], out_sb[:])

    nc.compile()
    return nc


def _prep(node_property_tensor, connectivity_tensor, bond_property_tensor,
          property_filters, bond_filters, y_dev=True):
    x = np.asarray(node_property_tensor, dtype=np.float32)
    conn = np.asarray(connectivity_tensor, dtype=np.float32)
    bp = np.asarray(bond_property_tensor, dtype=np.float32)
    pf = np.asarray(property_filters, dtype=np.float32)
    bf = np.asarray(bond_filters, dtype=np.float32)

    W = pf * bf[:, :, 0:1]                                # (O, F, I)
    wr = np.ascontiguousarray(W.transpose(2, 1, 0).reshape(IN_DEPTH, F * OUT_DEPTH))
    bf2 = np.ascontiguousarray(bf[:, :, 1:3].reshape(OUT_DEPTH, KB).T)  # (24, O)

    # conn packed per core: [p, (f, nb, a)] so each k-chunk is a contiguous
    # [128, AS] block in DRAM (16 KB/partition per 32-chunk DMA batch)
    # conn[a, n=nb*128+p, f] -> packed[p, f, nb, a]
    connb = conn.astype(NP_BF16)                          # (A, A, F)
    cview = connb.reshape(A, NBLK, KP, F)                 # [a, nb, p, f]
    cpack = cview.transpose(2, 3, 1, 0)                   # [p, f, nb, a]

    common = {}
    if y_dev:
        common["xw"] = np.ascontiguousarray(
            np.concatenate([x.T, wr], axis=1)).astype(NP_BF16)  # (64, A + F*O)
    else:
        xb = x.astype(NP_BF16).astype(np.float32)
        wrb = wr.astype(NP_BF16).astype(np.float32)
        Y = (xb @ wrb).reshape(A, F, OUT_DEPTH)           # [n, f, o]
        ypack = Y.transpose(1, 0, 2).reshape(F, NBLK, KP, OUT_DEPTH)  # [f, nb, p, o]
        common["ypack"] = np.ascontiguousarray(
            ypack.transpose(2, 0, 1, 3).reshape(KP, KC * OUT_DEPTH)).astype(NP_BF16)

    in_maps = []
    for c in range(NCORES):
        sl = slice(c * AS, (c + 1) * AS)
        bond_tc = bp[sl].reshape(AS, KB).T                # (24, AS)
        in_maps.append({
            "conn_t": np.ascontiguousarray(
                cpack[:, :, :, sl].reshape(KP, KC * AS)),
            "bpack": np.ascontiguousarray(
                np.concatenate([bond_tc, bf2], axis=1)),  # (24, AS + O)
            **common,
        })
    return in_maps


def kernel(node_property_tensor, connectivity_tensor, bond_property_tensor,
           property_filters, bond_filters):
    in_maps = _prep(node_property_tensor, connectivity_tensor,
                    bond_property_tensor, property_filters, bond_filters)

    if "nc" not in _cache:
        _cache["nc"] = _build_nc()
    nc = _cache["nc"]

    res = run_bass_kernel_spmd(nc, in_maps, core_ids=list(range(NCORES)))

    out = np.empty((A, OUT_DEPTH), dtype=np.float32)
    for c in range(NCORES):
        out[c * AS:(c + 1) * AS, :] = res.results[c]["out_t"].T
    return out
